# revision 20
# baseline (speedup 1.0000x reference)
"""MOLELinear (mixture-of-linear-experts) Trainium2 kernel.

Math (per group g): out_g = x_g @ (sum_e c[g,e] W_e + W_sh).T + (sum_e c[g,e] b_e + b_sh)

Sharding: data-parallel over the 32 groups -> 4 groups (8192 tokens) per core,
expert weights replicated. Host does layout-only prep (transpose / stacking /
fp16 rounding); all arithmetic of the reference runs on device.

Device plan per core (all fp16 data path, fp32 accumulation in PSUM):
  - DMA in: xT [512, 8192] fp16 (x shard transposed), wall [128, 9, 2048] fp16
    (shared + 8 experts, k-tile-major free layout), small coefficient/bias
    tensors, cdiag [128, 9, 128] (scaled identity matrices for group 0).
  - Group-0 weight mix on PE during the weight-DMA window:
    psum[:, kt] += diag(c_j) @ W_j[kt]  (keeps PE warm, fp32 accumulation).
  - Groups 1-3 weight mix on DVE: fp16 scalar_tensor_tensor FMA chains,
    FD=2048 per op (8 ops per group).
  - Mixed biases transposed on PE: mbT[ot][o,g] = sum_j ball[j,o] cx[j,g].
  - Main GEMM: stationary = mixed weight subtile [128k,128o], moving = xT
    slice [128k,512t]; psum [128 o, 2048 t] accumulates 4 k-tiles.
  - Drain on ScalarE via ACTIVATE(Identity, bias=mbT column): PSUM->SBUF fp16
    with the per-partition bias add fused in. DMA out on GpSimd (SWDGE) so
    output stores don't contend with the input DMA issue queue.
"""
import numpy as np

import concourse.bacc as bacc
import concourse.mybir as mybir
from concourse.alu_op_type import AluOpType
from concourse.tile import TileContext
from concourse.bass_utils import run_bass_kernel_spmd

N_CORES = 8
IN_F = 512
OUT_F = 512
N_EXPERTS = 8
N_GROUPS = 32
TOK_PER_GROUP = 2048
G_PER_CORE = N_GROUPS // N_CORES           # 4
TOK_PER_CORE = G_PER_CORE * TOK_PER_GROUP  # 8192
KT = IN_F // 128                           # 4 k-tiles
OT = OUT_F // 128                          # 4 out-feature tiles
NW = N_EXPERTS + 1                         # 9: shared weight first, then experts
F32 = mybir.dt.float32
F32R = mybir.dt.float32r
F16 = mybir.dt.float16
AF = mybir.ActivationFunctionType

_CACHE = {}


def _build():
    nc = bacc.Bacc(trn_type="TRN2")
    xT = nc.dram_tensor("xT", (IN_F, TOK_PER_CORE), F16, kind="ExternalInput")
    wall = nc.dram_tensor("wall", (128, NW, KT * OUT_F), F16, kind="ExternalInput")
    cdiag = nc.dram_tensor("cdiag", (128, NW, 128), F16, kind="ExternalInput")
    cb = nc.dram_tensor("cb", (128, G_PER_CORE * N_EXPERTS), F32, kind="ExternalInput")
    cx = nc.dram_tensor("cx", (NW, G_PER_CORE), F32R, kind="ExternalInput")
    ball = nc.dram_tensor("ball", (NW, OUT_F), F32R, kind="ExternalInput")
    outT = nc.dram_tensor("outT", (OUT_F, TOK_PER_CORE), F16, kind="ExternalOutput")

    with TileContext(nc) as tc:
        with (
            tc.tile_pool(name="smallp", bufs=1) as smallp,
            tc.tile_pool(name="wallp", bufs=1) as wallp,
            tc.tile_pool(name="wmp", bufs=1) as wmp,
            tc.tile_pool(name="xp", bufs=1) as xp,
            tc.tile_pool(name="ocp", bufs=4) as ocp,
        ):
            # ---- DMA issue order tuned so that: mixing chains start early
            # (cbt + expert 0 up front), wm0 (needs ALL experts) and xg0 land
            # at about the same time, later groups' x arrives last.
            cxt = smallp.tile([NW, G_PER_CORE], F32R, tag="cx")
            nc.sync.dma_start(cxt[:], cx[:])
            cbt = smallp.tile([128, G_PER_CORE * N_EXPERTS], F32, tag="cb")
            nc.sync.dma_start(cbt[:], cb[:])

            # expert weights in 4 merged transfers (fewer issue/fixed costs);
            # Tile range-deps let readers of early slices proceed
            walls = wallp.tile([128, NW * KT * OUT_F], F16, tag="walls")

            def wsl(j, c0=0, c1=KT * OUT_F):
                return walls[:, j * KT * OUT_F + c0 : j * KT * OUT_F + c1]

            def wall_dma(j0, j1):
                nc.sync.dma_start(
                    walls[:, j0 * KT * OUT_F : j1 * KT * OUT_F].rearrange(
                        "p (j c) -> p j c", j=j1 - j0
                    ),
                    wall[:, j0:j1, :],
                )

            wall_dma(0, 2)  # shared + expert 0
            cdt = smallp.tile([128, NW * 128], F16, tag="cdt")
            nc.sync.dma_start(cdt[:].rearrange("p (e m) -> p e m", e=NW), cdiag[:])
            ballt = smallp.tile([NW, OUT_F], F32R, tag="ball")
            nc.sync.dma_start(ballt[:], ball[:])
            wall_dma(2, 5)  # experts 1-3
            wall_dma(5, 8)  # experts 4-6

            # group-0 x per k-tile, straddling the last expert so wm0 and x
            # arrive together; the first GEMM phase is gated by both
            xg0t = [
                xp.tile([128, TOK_PER_GROUP], F16, tag=f"x0k{kt}", name=f"x0k{kt}")
                for kt in range(KT)
            ]
            nc.sync.dma_start(xg0t[0][:], xT[0:128, 0:TOK_PER_GROUP])
            nc.sync.dma_start(xg0t[1][:], xT[128:256, 0:TOK_PER_GROUP])
            wall_dma(8, 9)  # expert 7
            nc.sync.dma_start(xg0t[2][:], xT[256:384, 0:TOK_PER_GROUP])
            nc.sync.dma_start(xg0t[3][:], xT[384:512, 0:TOK_PER_GROUP])

            xg = [None]
            for g in range(1, G_PER_CORE):
                t = xp.tile([128, KT * TOK_PER_GROUP], F16, tag=f"x{g}", name=f"x{g}")
                nc.sync.dma_start(
                    t[:].rearrange("p (kt t) -> p kt t", kt=KT),
                    xT[:, g * TOK_PER_GROUP : (g + 1) * TOK_PER_GROUP].rearrange(
                        "(kt p) t -> p kt t", p=128
                    ),
                )
                xg.append(t)

            def xslice(g, kt, tci):
                if g == 0:
                    return xg0t[kt][:, tci * 512 : (tci + 1) * 512]
                return xg[g][
                    :,
                    kt * TOK_PER_GROUP + tci * 512 : kt * TOK_PER_GROUP + (tci + 1) * 512,
                ]

            wm = [
                wmp.tile([128, KT * OUT_F], F16, tag=f"wm{g}", name=f"wm{g}")
                for g in range(G_PER_CORE)
            ]

            with tc.tile_pool(name="ps", bufs=2, space="PSUM") as ps:
                # ---- mixed biases (one bank, freed early):
                # mbT2[o', ot*4+g] = sum_j ball[j, ot*128+o'] cx[j, g]
                pb = ps.tile([128, OT * G_PER_CORE], F32, tag="ps")
                for ot in range(OT):
                    nc.tensor.matmul(
                        pb[:, ot * G_PER_CORE : (ot + 1) * G_PER_CORE],
                        ballt[:, ot * 128 : (ot + 1) * 128],
                        cxt[:],
                        start=True,
                        stop=True,
                    )
                mbT2 = smallp.tile([128, OT * G_PER_CORE], F32, tag="mbT2")
                nc.scalar.copy(mbT2[:], pb[:])

                # ---- groups 1-3 weight mix on DVE, two-step per term:
                # tensor_scalar (4x mode) then tensor_tensor (2x mode)
                for g in (1, 2, 3):
                    for e in range(N_EXPERTS):
                        tmp = wmp.tile(
                            [128, KT * OUT_F], F16, tag="tmp", name="tmp", bufs=2
                        )
                        nc.vector.tensor_scalar(
                            tmp[:],
                            wsl(e + 1),
                            cbt[:, g * N_EXPERTS + e : g * N_EXPERTS + e + 1],
                            None,
                            AluOpType.mult,
                        )
                        nc.vector.tensor_tensor(
                            wm[g][:],
                            tmp[:],
                            wsl(0) if e == 0 else wm[g][:],
                            AluOpType.add,
                        )

                # ---- group-0 weight mix on PE (runs while weights stream in);
                # last expert's matmuls interleaved with per-k-tile casts so
                # wm[0] is ready ~1.5us after the final weight slice lands
                pm = ps.tile([128, KT * OUT_F], F32, tag="ps")
                for j in range(NW - 1):
                    for kt in range(KT):
                        nc.tensor.matmul(
                            pm[:, kt * OUT_F : (kt + 1) * OUT_F],
                            cdt[:, j * 128 : (j + 1) * 128],
                            wsl(j, kt * OUT_F, (kt + 1) * OUT_F),
                            start=(j == 0),
                            stop=False,
                        )
                j = NW - 1
                for kt in range(KT):
                    nc.tensor.matmul(
                        pm[:, kt * OUT_F : (kt + 1) * OUT_F],
                        cdt[:, j * 128 : (j + 1) * 128],
                        wsl(j, kt * OUT_F, (kt + 1) * OUT_F),
                        start=False,
                        stop=True,
                    )
                    nc.scalar.copy(
                        wm[0][:, kt * OUT_F : (kt + 1) * OUT_F],
                        pm[:, kt * OUT_F : (kt + 1) * OUT_F],
                    )

                # ---- main GEMM ----
                for g in range(G_PER_CORE):
                    for ot in range(OT):
                        pt = ps.tile([128, TOK_PER_GROUP], F32, tag="ps")
                        for kt in range(KT):
                            lhsT = wm[g][
                                :, kt * OUT_F + ot * 128 : kt * OUT_F + (ot + 1) * 128
                            ]
                            for tci in range(TOK_PER_GROUP // 512):
                                nc.tensor.matmul(
                                    pt[:, tci * 512 : (tci + 1) * 512],
                                    lhsT,
                                    xslice(g, kt, tci),
                                    start=(kt == 0),
                                    stop=(kt == KT - 1),
                                )
                        oc = ocp.tile([128, TOK_PER_GROUP], F16, tag="oc")
                        bias_ap = mbT2[:, ot * G_PER_CORE + g : ot * G_PER_CORE + g + 1]
                        last = g == G_PER_CORE - 1 and ot == OT - 1
                        halves = 2 if last else 1
                        hw = TOK_PER_GROUP // halves
                        for h in range(halves):
                            nc.scalar.activation(
                                oc[:, h * hw : (h + 1) * hw],
                                pt[:, h * hw : (h + 1) * hw],
                                AF.Identity,
                                bias=bias_ap,
                                scale=1.0,
                            )
                            nc.scalar.dma_start(
                                outT[
                                    ot * 128 : (ot + 1) * 128,
                                    g * TOK_PER_GROUP + h * hw : g * TOK_PER_GROUP
                                    + (h + 1) * hw,
                                ],
                                oc[:, h * hw : (h + 1) * hw],
                            )
    nc.finalize()
    return nc


def kernel(x, coefficients, weight_experts, bias_experts, weight_shared, bias_shared, sizes):
    x = np.asarray(x)
    coefficients = np.asarray(coefficients, dtype=np.float32)
    weight_experts = np.asarray(weight_experts, dtype=np.float32)
    bias_experts = np.asarray(bias_experts, dtype=np.float32)
    weight_shared = np.asarray(weight_shared, dtype=np.float32)
    bias_shared = np.asarray(bias_shared, dtype=np.float32)

    if "nc" not in _CACHE:
        _CACHE["nc"] = _build()
    nc = _CACHE["nc"]

    # ---- host-side layout prep ----
    x16 = x.astype(np.float16)
    # wall[p, j, kt*512+o] = W_j^T[kt*128+p, o]; j=0 shared, j=1+e expert e
    wall_np = np.empty((128, NW, KT * OUT_F), np.float16)
    for j in range(NW):
        W = weight_shared if j == 0 else weight_experts[j - 1]
        arr = W.T.reshape(KT, 128, OUT_F).transpose(1, 0, 2).reshape(128, KT * OUT_F)
        wall_np[:, j, :] = arr.astype(np.float16)
    ball_np = np.empty((NW, OUT_F), np.float32)
    ball_np[0] = bias_shared
    ball_np[1:] = bias_experts

    in_maps = []
    for c in range(N_CORES):
        gs = slice(c * G_PER_CORE, (c + 1) * G_PER_CORE)
        cg = coefficients[gs]  # [4, 8]
        cb_np = np.broadcast_to(
            cg.reshape(1, -1), (128, G_PER_CORE * N_EXPERTS)
        ).copy()
        cx_np = np.empty((NW, G_PER_CORE), np.float32)
        cx_np[0] = 1.0
        cx_np[1:] = cg.T
        cd_np = np.zeros((128, NW, 128), np.float16)
        idx = np.arange(128)
        cd_np[idx, 0, idx] = 1.0
        for e in range(N_EXPERTS):
            cd_np[idx, 1 + e, idx] = np.float16(cg[0, e])
        xT_np = np.ascontiguousarray(
            x16[c * TOK_PER_CORE : (c + 1) * TOK_PER_CORE].T
        )
        in_maps.append(
            {
                "xT": xT_np,
                "wall": wall_np,
                "cdiag": cd_np,
                "cb": cb_np,
                "cx": cx_np,
                "ball": ball_np,
            }
        )

    res = run_bass_kernel_spmd(nc, in_maps, core_ids=list(range(N_CORES)))
    out = np.empty((N_CORES * TOK_PER_CORE, OUT_F), np.float32)
    for c in range(N_CORES):
        out[c * TOK_PER_CORE : (c + 1) * TOK_PER_CORE] = (
            np.asarray(res.results[c]["outT"]).T.astype(np.float32)
        )
    return out


# revision 21
# speedup vs baseline: 1.1611x; 1.1611x over previous
"""MOLELinear (mixture-of-linear-experts) Trainium2 kernel.

Math (per group g): out_g = x_g @ (sum_e c[g,e] W_e + W_sh).T + (sum_e c[g,e] b_e + b_sh)

Sharding: data-parallel over the 32 groups -> 4 groups (8192 tokens) per core,
expert weights replicated. Host does layout-only prep (transpose / stacking /
fp16 rounding); all arithmetic of the reference runs on device.

Device plan per core (all fp16 data path, fp32 accumulation in PSUM):
  - Weights arrive in two o-halves (wallA = out-feature tiles 0-1, wallB =
    tiles 2-3), each half expert-contiguous, so the first GEMM phases start
    after only half the weight bytes have landed.
  - Group-0 weight mix on PE during the weight-DMA window via scaled-identity
    diagonal matmuls (fp32 PSUM accumulation), per o-half.
  - Groups 1-3 weight mix on DVE as per-half FMA chains:
    tensor_scalar (4x mode) + tensor_tensor (2x mode), FD=1024.
  - Mixed biases transposed on PE: mbT2[o', ot*4+g] = sum_j ball[j,o]cx[j,g].
  - Main GEMM: stationary = mixed-weight subtile [128k,128o], moving = xT
    slice [128k,512t]; psum [128 o', 1024 t] (2 banks) over 4 k-tiles.
    Phase order interleaves groups by weight half so DVE mixing keeps up.
  - Drain on ScalarE via ACTIVATE(Identity, bias=mbT2 column) -> fp16 SBUF;
    output stores on the second HWDGE ring (nc.scalar).
"""
import numpy as np

import concourse.bacc as bacc
import concourse.mybir as mybir
from concourse.alu_op_type import AluOpType
from concourse.tile import TileContext
from concourse.bass_utils import run_bass_kernel_spmd

N_CORES = 8
IN_F = 512
OUT_F = 512
N_EXPERTS = 8
N_GROUPS = 32
TOK_PER_GROUP = 2048
G_PER_CORE = N_GROUPS // N_CORES           # 4
TOK_PER_CORE = G_PER_CORE * TOK_PER_GROUP  # 8192
KT = IN_F // 128                           # 4 k-tiles
OT = OUT_F // 128                          # 4 out-feature tiles
NW = N_EXPERTS + 1                         # 9: shared weight first, then experts
HALF = KT * OUT_F // 2                     # 1024 columns per o-half
F32 = mybir.dt.float32
F32R = mybir.dt.float32r
F16 = mybir.dt.float16
AF = mybir.ActivationFunctionType

_CACHE = {}


def _build():
    nc = bacc.Bacc(trn_type="TRN2")
    xT = nc.dram_tensor("xT", (IN_F, TOK_PER_CORE), F16, kind="ExternalInput")
    wallA = nc.dram_tensor("wallA", (128, NW, HALF), F16, kind="ExternalInput")
    wallB = nc.dram_tensor("wallB", (128, NW, HALF), F16, kind="ExternalInput")
    cdiag = nc.dram_tensor("cdiag", (128, NW, 128), F16, kind="ExternalInput")
    cb = nc.dram_tensor("cb", (128, G_PER_CORE * N_EXPERTS), F32, kind="ExternalInput")
    cx = nc.dram_tensor("cx", (NW, G_PER_CORE), F32R, kind="ExternalInput")
    ball = nc.dram_tensor("ball", (NW, OUT_F), F32R, kind="ExternalInput")
    outT = nc.dram_tensor("outT", (OUT_F, TOK_PER_CORE), F16, kind="ExternalOutput")

    with TileContext(nc) as tc:
        with (
            tc.tile_pool(name="smallp", bufs=1) as smallp,
            tc.tile_pool(name="wallp", bufs=1) as wallp,
            tc.tile_pool(name="wmp", bufs=1) as wmp,
            tc.tile_pool(name="xp", bufs=1) as xp,
            tc.tile_pool(name="ocp", bufs=4) as ocp,
        ):
            # ---- front DMAs: smalls, first weight half, g0 x, second half ----
            cxt = smallp.tile([NW, G_PER_CORE], F32R, tag="cx")
            nc.sync.dma_start(cxt[:], cx[:])
            cbt = smallp.tile([128, G_PER_CORE * N_EXPERTS], F32, tag="cb")
            nc.sync.dma_start(cbt[:], cb[:])
            cdt = smallp.tile([128, NW * 128], F16, tag="cdt")
            nc.sync.dma_start(cdt[:].rearrange("p (e m) -> p e m", e=NW), cdiag[:])

            wallsA = wallp.tile([128, NW * HALF], F16, tag="wallsA")
            nc.sync.dma_start(
                wallsA[:].rearrange("p (e c) -> p e c", e=NW), wallA[:]
            )
            ballt = smallp.tile([NW, OUT_F], F32R, tag="ball")
            nc.sync.dma_start(ballt[:], ball[:])

            xg0t = [
                xp.tile([128, TOK_PER_GROUP], F16, tag=f"x0k{kt}", name=f"x0k{kt}")
                for kt in range(KT)
            ]
            for kt in range(KT):
                nc.sync.dma_start(
                    xg0t[kt][:], xT[kt * 128 : (kt + 1) * 128, 0:TOK_PER_GROUP]
                )

            wallsB = wallp.tile([128, NW * HALF], F16, tag="wallsB")
            nc.sync.dma_start(
                wallsB[:].rearrange("p (e c) -> p e c", e=NW), wallB[:]
            )

            xg = [None]
            for g in range(1, G_PER_CORE):
                t = xp.tile([128, KT * TOK_PER_GROUP], F16, tag=f"x{g}", name=f"x{g}")
                nc.sync.dma_start(
                    t[:].rearrange("p (kt t) -> p kt t", kt=KT),
                    xT[:, g * TOK_PER_GROUP : (g + 1) * TOK_PER_GROUP].rearrange(
                        "(kt p) t -> p kt t", p=128
                    ),
                )
                xg.append(t)

            walls = [wallsA, wallsB]

            def wsl(h, j, c0, c1):
                return walls[h][:, j * HALF + c0 : j * HALF + c1]

            def xslice(g, kt, tci):
                if g == 0:
                    return xg0t[kt][:, tci * 512 : (tci + 1) * 512]
                return xg[g][
                    :,
                    kt * TOK_PER_GROUP + tci * 512 : kt * TOK_PER_GROUP + (tci + 1) * 512,
                ]

            # wm layout per group: column = ot*512 + kt*128 + o'  (o-major)
            wm = [
                wmp.tile([128, KT * OUT_F], F16, tag=f"wm{g}", name=f"wm{g}")
                for g in range(G_PER_CORE)
            ]

            with (
                tc.tile_pool(name="psd", bufs=1, space="PSUM") as psd,
                tc.tile_pool(name="ps", bufs=3, space="PSUM") as ps,
            ):
                # ---- mixed biases (rides the psd slot, freed early) ----
                pb = psd.tile([128, HALF], F32, tag="psd", name="pb")
                for ot in range(OT):
                    nc.tensor.matmul(
                        pb[:, ot * G_PER_CORE : (ot + 1) * G_PER_CORE],
                        ballt[:, ot * 128 : (ot + 1) * 128],
                        cxt[:],
                        start=True,
                        stop=True,
                    )
                mbT2 = smallp.tile([128, OT * G_PER_CORE], F32, tag="mbT2")
                nc.scalar.copy(mbT2[:], pb[:, 0 : OT * G_PER_CORE])

                # ---- group-0 weight mix on PE, one o-half at a time ----
                for h in range(2):
                    pm = psd.tile([128, HALF], F32, tag="psd", name=f"pm{h}")
                    for otl in range(2):
                        for j in range(NW):
                            nc.tensor.matmul(
                                pm[:, otl * 512 : (otl + 1) * 512],
                                cdt[:, j * 128 : (j + 1) * 128],
                                wsl(h, j, otl * 512, (otl + 1) * 512),
                                start=(j == 0),
                                stop=(j == NW - 1),
                            )
                        nc.scalar.copy(
                            wm[0][
                                :, h * HALF + otl * 512 : h * HALF + (otl + 1) * 512
                            ],
                            pm[:, otl * 512 : (otl + 1) * 512],
                        )

                # ---- groups 1-3 weight mix on DVE, per half, interleaved so
                # each chain completes just before its GEMM phases need it
                def mix_chain(g, h):
                    for e in range(N_EXPERTS):
                        tmp = wmp.tile([128, HALF], F16, tag="tmp", name="tmp", bufs=2)
                        nc.vector.tensor_scalar(
                            tmp[:],
                            wsl(h, e + 1, 0, HALF),
                            cbt[:, g * N_EXPERTS + e : g * N_EXPERTS + e + 1],
                            None,
                            AluOpType.mult,
                        )
                        nc.vector.tensor_tensor(
                            wm[g][:, h * HALF : (h + 1) * HALF],
                            tmp[:],
                            wsl(h, 0, 0, HALF)
                            if e == 0
                            else wm[g][:, h * HALF : (h + 1) * HALF],
                            AluOpType.add,
                        )

                for g, h in ((1, 0), (2, 0), (1, 1), (2, 1), (3, 0), (3, 1)):
                    mix_chain(g, h)

                # ---- main GEMM: phases interleaved by weight half ----
                phase_order = [
                    (0, 0), (0, 1), (0, 2), (0, 3),
                    (1, 0), (1, 1), (2, 0), (2, 1),
                    (1, 2), (1, 3), (2, 2), (2, 3),
                    (3, 0), (3, 1), (3, 2), (3, 3),
                ]
                n_phase = len(phase_order)
                for pi, (g, ot) in enumerate(phase_order):
                    oc = ocp.tile([128, TOK_PER_GROUP], F16, tag="oc", name="oc")
                    bias_ap = mbT2[:, ot * G_PER_CORE + g : ot * G_PER_CORE + g + 1]
                    last = pi == n_phase - 1
                    for th in range(2):
                        pt = ps.tile([128, 1024], F32, tag="ps", name="pt")
                        for kt in range(KT):
                            lhsT = wm[g][
                                :, ot * 512 + kt * 128 : ot * 512 + (kt + 1) * 128
                            ]
                            for tci in range(2):
                                nc.tensor.matmul(
                                    pt[:, tci * 512 : (tci + 1) * 512],
                                    lhsT,
                                    xslice(g, kt, th * 2 + tci),
                                    start=(kt == 0),
                                    stop=(kt == KT - 1),
                                )
                        nc.scalar.activation(
                            oc[:, th * 1024 : (th + 1) * 1024],
                            pt[:],
                            AF.Identity,
                            bias=bias_ap,
                            scale=1.0,
                        )
                        if last:
                            nc.scalar.dma_start(
                                outT[
                                    ot * 128 : (ot + 1) * 128,
                                    g * TOK_PER_GROUP
                                    + th * 1024 : g * TOK_PER_GROUP
                                    + (th + 1) * 1024,
                                ],
                                oc[:, th * 1024 : (th + 1) * 1024],
                            )
                    if not last:
                        nc.scalar.dma_start(
                            outT[
                                ot * 128 : (ot + 1) * 128,
                                g * TOK_PER_GROUP : (g + 1) * TOK_PER_GROUP,
                            ],
                            oc[:],
                        )
    nc.finalize()
    return nc


def kernel(x, coefficients, weight_experts, bias_experts, weight_shared, bias_shared, sizes):
    x = np.asarray(x)
    coefficients = np.asarray(coefficients, dtype=np.float32)
    weight_experts = np.asarray(weight_experts, dtype=np.float32)
    bias_experts = np.asarray(bias_experts, dtype=np.float32)
    weight_shared = np.asarray(weight_shared, dtype=np.float32)
    bias_shared = np.asarray(bias_shared, dtype=np.float32)

    if "nc" not in _CACHE:
        _CACHE["nc"] = _build()
    nc = _CACHE["nc"]

    # ---- host-side layout prep ----
    x16 = x.astype(np.float16)
    # per expert j: X[p, ot, kt, o'] = W_j^T[kt*128+p, ot*128+o']
    # wallA = out-feature tiles 0-1, wallB = tiles 2-3 (each [128, 9, 1024])
    wallA_np = np.empty((128, NW, HALF), np.float16)
    wallB_np = np.empty((128, NW, HALF), np.float16)
    for j in range(NW):
        W = weight_shared if j == 0 else weight_experts[j - 1]
        X = (
            W.T.reshape(KT, 128, OT, 128)
            .transpose(1, 2, 0, 3)
            .astype(np.float16)
        )  # [p, ot, kt, o']
        wallA_np[:, j, :] = X[:, 0:2].reshape(128, HALF)
        wallB_np[:, j, :] = X[:, 2:4].reshape(128, HALF)
    ball_np = np.empty((NW, OUT_F), np.float32)
    ball_np[0] = bias_shared
    ball_np[1:] = bias_experts

    in_maps = []
    for c in range(N_CORES):
        gs = slice(c * G_PER_CORE, (c + 1) * G_PER_CORE)
        cg = coefficients[gs]  # [4, 8]
        cb_np = np.broadcast_to(
            cg.reshape(1, -1), (128, G_PER_CORE * N_EXPERTS)
        ).copy()
        cx_np = np.empty((NW, G_PER_CORE), np.float32)
        cx_np[0] = 1.0
        cx_np[1:] = cg.T
        cd_np = np.zeros((128, NW, 128), np.float16)
        idx = np.arange(128)
        cd_np[idx, 0, idx] = 1.0
        for e in range(N_EXPERTS):
            cd_np[idx, 1 + e, idx] = np.float16(cg[0, e])
        xT_np = np.ascontiguousarray(
            x16[c * TOK_PER_CORE : (c + 1) * TOK_PER_CORE].T
        )
        in_maps.append(
            {
                "xT": xT_np,
                "wallA": wallA_np,
                "wallB": wallB_np,
                "cdiag": cd_np,
                "cb": cb_np,
                "cx": cx_np,
                "ball": ball_np,
            }
        )

    res = run_bass_kernel_spmd(nc, in_maps, core_ids=list(range(N_CORES)))
    out = np.empty((N_CORES * TOK_PER_CORE, OUT_F), np.float32)
    for c in range(N_CORES):
        out[c * TOK_PER_CORE : (c + 1) * TOK_PER_CORE] = (
            np.asarray(res.results[c]["outT"]).T.astype(np.float32)
        )
    return out
